# revision 39
# baseline (speedup 1.0000x reference)
"""PointNet feature-propagation module on 8 Trainium2 cores.

Reference computation (per batch):
  dist, idx = 3-NN of xyz1 (n=4096) in xyz2 (m=1024)
  dist clamped to [0, 1e-10]  -> interpolation weights are exactly w=1/3
  interp = sum_k w * points2[idx_k]                    (n, 512)
  feat = [interp, points1] @ W1^T -> BN -> ReLU        (n, 256)
  feat = feat @ W2^T -> BN -> ReLU                     (n, 256)
  out = feat^T                                         (256, n)
BN statistics are over (batch, n) across ALL 16 batches -> AllReduce.

Strategy (data-parallel, 2 batches/core), v2 software-pipelined:
  - 3-NN via threshold masks: per query n, tau = midpoint of the 3rd/4th
    smallest distance (top-8 of negdist on DVE), then mask[m, n] selects
    the 3 nearest.  interp collapses into y1a^T = Z @ mask with
    Z = points2 @ (0.5*w*W1a)^T (plus a colsum correction for +-1 masks).
  - Distances on the PE with fp32-grade precision via 3-term fp16 splits.
  - Streamed pipeline: both batches' distance passes run up front
    (interleaved groups, DVE-bound) with batch 0's mask phase lagging one
    tau-group behind; batch 1's mask phase then runs purely on PE/ACT.
    The BN-normalize / layer-2 / BN2 / output tail of iteration i is
    deferred into iterations i+1 and i+2 via an out-of-order piece queue,
    so both AllReduce latencies hide under independent work and every
    collective is emitted only once its inputs are computed (a waiting
    collective parks the Pool sequencer).
  - Mask-matmuls run in fp8e4m3 DoubleRow (masks are exact in fp8; Z is
    split hi+lo at 8x scale, rescaled at PSUM evacuation).
  - relu(s1*y1+t1) = max(s1*y1,-t1) + t1: the max is one DVE 2x-mode op
    and +t1 folds into c = W2@t1 applied at the y2 evacuation.
  - BN scale/shift: Exp(-0.5*Ln(var+eps)) on ACT + Newton steps on DVE.
"""
import numpy as np

import concourse.bass as bass
import concourse.bacc as bacc
import concourse.tile as tile
import concourse.mybir as mybir
import concourse.bass_utils as bass_utils

F32 = mybir.dt.float32
F16 = mybir.dt.float16
F8 = mybir.dt.float8e4
Z8 = 8.0          # fp8 pre-scale for the Z hi/lo split
AF = mybir.ActivationFunctionType
ALU = mybir.AluOpType

N_CORES = 8
B_PER_CORE = 2
N = 4096          # query points per batch
M = 1024          # source points per batch
C1 = 256          # points1 channels
C2 = 512          # points2 channels
O = 256           # conv output channels
NT = N // 128     # 32 n-tiles
MT = M // 128     # 8 m-tiles
H = 512           # n-chunk for phase E
NH = N // H       # 8 chunks
KROWS = 24        # K rows of the distance matmuls (21 data + 3 tau)
KD = 21           # rows without tau
EPS_BN = 1e-5
# (b, h) chunks whose mask is built on DVE ({0,2} masks, no colsum bias);
# the rest are built on ACT (Sign {-1,+1} masks, colsum bias at evac).
# DVE is the backlog engine (top-8 + bn_stats queue deep): a DVE mask
# chunk would park the whole maskmul pipeline behind that backlog.
DVE_MASK = set()

_PROGRAM_CACHE = {}


def _split3(x32):
    """3-term fp16 split: x ~ a+b+c with ~2^-33 relative error."""
    a = x32.astype(np.float16)
    r1 = x32 - a.astype(np.float32)
    b = r1.astype(np.float16)
    r2 = r1 - b.astype(np.float32)
    c = r2.astype(np.float16)
    return a, b, c


def _build_sides(x1, x2):
    """Build the K-row operands for the two distance matmuls.

    negdist'[n, m] = 2*x1[n]@x2[m] - |x2[m]|^2   (|x1|^2 dropped: constant
    per n, does not affect the per-n ranking over m).
    """
    n, m = x1.shape[0], x2.shape[0]
    s1 = np.zeros((KROWS, n), np.float16)
    s2 = np.zeros((KROWS, m), np.float16)
    for ci in range(3):
        u, v, w = _split3(x1[:, ci].astype(np.float32))
        a, b, c = _split3(x2[:, ci].astype(np.float32))
        r = 6 * ci
        s1[r + 0], s2[r + 0] = 2.0 * u, a
        s1[r + 1], s2[r + 1] = 2.0 * v, a
        s1[r + 2], s2[r + 2] = 2.0 * u, b
        s1[r + 3], s2[r + 3] = 2.0 * v, b
        s1[r + 4], s2[r + 4] = 2.0 * w, a
        s1[r + 5], s2[r + 5] = 2.0 * u, c
    x2f = x2.astype(np.float32)
    S = (x2f[:, 0] * x2f[:, 0] + x2f[:, 1] * x2f[:, 1]) + x2f[:, 2] * x2f[:, 2]
    sa, sb, sc = _split3(S)
    s1[18:21] = -1.0
    s2[18], s2[19], s2[20] = sa, sb, sc
    s2[21:23] = -1.0
    return s1, s2


class _Body:
    """Emits one repeat-iteration; carries deferred tail pieces forward."""

    def __init__(self, nc, tc, P, D, C, rep):
        self.nc, self.tc, self.P, self.D, self.C = nc, tc, P, D, C
        self.rep = rep

    # ---------- pieces infrastructure ----------
    def pop(self, pending, kind, n=1):
        q = pending[kind]
        for _ in range(n):
            if q:
                q.pop(0)()

    # ---------- phases ----------
    def emit(self, pending):
        nc, P, D, C = self.nc, self.P, self.D, self.C
        B = B_PER_CORE
        st = {}
        # ---- inputs ----
        x1s_l, x2s_l, p1T_l, z_l, cs_l = [], [], [], [], []
        for b in range(B):
            x1s = P["inp"].tile([KROWS, N], F16, tag="x1s")
            nc.sync.dma_start(x1s[0:KD, :], D["x1s"].ap()[b][0:KD, :])
            x2s = P["inp"].tile([KROWS, M], F16, tag="x2s")
            nc.sync.dma_start(x2s[:], D["x2s"].ap()[b])
            p1T = P["inp"].tile([128, C1 // 128, N], F16, tag="p1T")
            nc.sync.dma_start(p1T[:], D["p1T"].ap()[b].rearrange(
                "(k p) n -> p k n", p=128))
            p2T = P["p2pool"].tile([128, C2 // 128, M], F16, tag="p2T")
            nc.sync.dma_start(p2T[:], D["p2T"].ap()[b].rearrange(
                "(k p) m -> p k m", p=128))
            cs_sb = P["small"].tile([128, 2], F32, tag="cs_sb")
            nc.sync.dma_start(cs_sb[:], D["csb"].ap()[b])
            x1s_l.append(x1s)
            x2s_l.append(x2s)
            p1T_l.append(p1T)
            cs_l.append(cs_sb)
            # ---- Z = points2 @ (0.5*w*W1a)^T, stored as fp8 hi+lo of
            # 8*Z for DoubleRow maskmuls (masks are exact in fp8; the
            # split recovers ~2^-8 relative accuracy on Z) ----
            zhi = P["zpool"].tile([128, MT, O], F8, tag="zhi",
                                  name=f"zhi{b}")
            zlo = P["zpool"].tile([128, MT, O], F8, tag="zlo",
                                  name=f"zlo{b}")
            z_l.append((zhi, zlo))
            for mt in range(MT):
                z_ps = P["psB"].tile([128, O], F32, tag="scr")
                for kt in range(C2 // 128):
                    nc.tensor.matmul(
                        z_ps[:], p2T[:, kt, mt * 128:(mt + 1) * 128],
                        C["zw"][:, kt, :],
                        start=(kt == 0), stop=(kt == C2 // 128 - 1))
                nc.scalar.mul(zhi[:, mt, :], z_ps[:], Z8)
                nc.vector.scalar_tensor_tensor(
                    zlo[:, mt, :], z_ps[:], Z8, zhi[:, mt, :],
                    ALU.mult, ALU.subtract)
        st.update(x1s_l=x1s_l, x2s_l=x2s_l, p1T_l=p1T_l, z_l=z_l, cs_l=cs_l)

        # ---- streamed pipeline ----
        # Both batches' distance passes run up front (interleaved groups,
        # DVE-bound); batch 0's E chunks lag one group behind its tau so
        # they overlap the top-8 stream; batch 1's E chunks then run in a
        # "pure" PE/ACT phase with no DVE dependency left.  tau only
        # depends on its own n-tiles' top-8, not the whole batch.
        y1_l = [None, None]
        y1_l[0] = P["ybig"].tile([128, 2, N], F16, tag="y1", name="y1a")
        y1_l[1] = P["ybig"].tile([128, 2, N], F16, tag="y1", name="y1b")
        bn1_strip = P["stats"].tile([128, 2, B * NH * 6], F32, tag="bn1")
        # Out-of-order pop schedule (see make_tail): "late" = s2t2+OUT of
        # iteration i-2 (collectives long done — zero stall, and OUT's ACT
        # ops are warm-up cover); "main" = s1t1+NME+R2 of i-1.  s1t1 pops
        # ~35us in (R1 latency hidden); all 8 NME(b0) pieces pop before
        # E(b1, h0) (y1 slot reuse at bufs=3 would deadlock otherwise);
        # R2 fires mid-body so the two AllReduces never serialize.
        strip_l = []
        for b in range(B):
            strip = P["small"].tile([128, NT, 8], F32, tag="strip",
                                    name=f"strip{b}")
            strip_l.append(strip)
        for g in range(4):
            for b in range(B):
                self.pass1_group8(b, x1s_l[b], x2s_l[b], strip_l[b], g)
                self.tau_group(x1s_l[b], strip_l[b], g)
            if g > 0:
                for h in (2 * g - 2, 2 * g - 1):
                    self.phaseE_chunk(0, h, x1s_l[0], x2s_l[0], p1T_l[0],
                                      z_l[0], cs_l[0], y1_l[0])
            if self.rep >= 2:  # late pieces lag two bodies
                self.pop(pending, "late", (3, 3, 3, 0)[g])
            # R1(i-1) fires at g1 (inputs landed just after the boundary);
            # s1t1 at g3 (~28us later, wire latency hidden).
            self.pop(pending, "main", (0, 1, 0, 4)[g])
        for h in (NH - 2, NH - 1):
            self.phaseE_chunk(0, h, x1s_l[0], x2s_l[0], p1T_l[0],
                              z_l[0], cs_l[0], y1_l[0])
        self.pop(pending, "main", 5)  # rest of NM(b0): all 8 before E(b1)
        # bn1 stats all emitted here (pure phase): keeping them out of the
        # g-loop keeps the DVE top-8 backlog short, so the BN collectives'
        # inputs are never stuck behind it.
        for h in range(NH):
            self.phaseE_chunk(1, h, x1s_l[1], x2s_l[1], p1T_l[1],
                              z_l[1], cs_l[1], y1_l[1])
            self.bn_chunk(bn1_strip, 0, h, y1_l[0])
            self.pop(pending, "main", (4, 4, 1, 1, 1, 1, 1, 0)[h])
        for h in range(NH):
            self.bn_chunk(bn1_strip, 1, h, y1_l[1])
        ar1in = self.bn_prep(bn1_strip, "bn1")

        # ---- build deferred tail pieces (R1 fires early next body) ----
        st.update(y1_l=y1_l, ar1in=ar1in)
        main, late = self.make_tail(st)
        pending["main"].extend(main)
        pending["late"].extend(late)
        return pending

    def pass1_group8(self, b, x1s, x2s, strip, g):
        """8 n-tiles of the distance pass: d1 halves + top8 + merge."""
        nc, P = self.nc, self.P
        for nt in range(8 * g, 8 * g + 8):
            s16 = P["small"].tile([128, 16], F32, tag="s16")
            for half in range(2):
                d1 = P["psA"].tile([128, 512], F32, tag="d1", name="d1")
                nc.tensor.matmul(
                    d1[:], x1s[0:KD, nt * 128:(nt + 1) * 128],
                    x2s[0:KD, half * 512:(half + 1) * 512],
                    start=True, stop=True)
                nc.vector.max(s16[:, half * 8:half * 8 + 8], d1[:])
            nc.vector.max(strip[:, nt, :], s16[:])

    def tau_group(self, x1s, strip, g):
        """tau = (v2+v3)/2 for 8 n-tiles -> 3-term split -> x1s rows 21:24.

        All elementwise work on Pool (DVE is the scarce engine); the
        [128, 8] -> [8, 128] transpose on the PE."""
        nc, P = self.nc, self.P
        gs = slice(8 * g, 8 * g + 8)
        tmat = P["small"].tile([128, 8], F32, tag="tmat",
                               name=f"tmat{g}")
        nc.vector.tensor_tensor(tmat[:], strip[:, gs, 2], strip[:, gs, 3],
                                ALU.add)
        nc.vector.tensor_scalar(tmat[:], tmat[:], 0.5, None, ALU.mult)
        tT_ps = P["psB"].tile([8, 128], F32, tag="scr", name=f"tTps{g}")
        nc.tensor.matmul(tT_ps[:], tmat[:], self.C["ident"][:],
                         is_transpose=True)
        # 2-term fp16 tau split: tau lives in the offset domain (|x1|^2
        # is not subtracted) so its magnitude can reach ~15; one fp16 term
        # leaves ~7e-3 absolute error vs a ~0.01 3rd/4th-NN gap.  Two
        # terms give ~4e-6 — safe — while saving a third of the chain.
        tT = P["small"].tile([8, 128], F32, tag="tT", name=f"tT{g}")
        nc.vector.tensor_copy(tT[:], tT_ps[:])
        th = P["small"].tile([8, 128], F16, tag="th", name=f"th{g}")
        nc.vector.tensor_copy(th[:], tT[:])
        r1 = P["small"].tile([8, 128], F32, tag="r1", name=f"r1{g}")
        nc.vector.tensor_tensor(r1[:], tT[:], th[:], ALU.subtract)
        tl = P["small"].tile([8, 128], F16, tag="tl", name=f"tl{g}")
        nc.vector.tensor_copy(tl[:], r1[:])
        for i, tsrc in enumerate((th, tl)):
            nc.gpsimd.dma_start(
                x1s[KD + i:KD + i + 1, 8 * g * 128:(8 * g + 8) * 128]
                .rearrange("a (q p) -> a q p", q=8, p=128),
                tsrc[:, :])

    def phaseE_chunk(self, b, h, x1s, x2s, p1T, z_sb, cs_sb, y1_sb):
        """mask + y1 accumulation for one (batch, h) chunk."""
        nc, P = self.nc, self.P
        hs = slice(h * H, (h + 1) * H)
        dve_mask = (b, h) in DVE_MASK
        zhi, zlo = z_sb
        py = P["psY"].tile([128, 2, H], F32, tag="psy")
        for mp in range(MT // 2):
            msk = P["masks"].tile([128, 2, H], F8, tag="msk",
                                  name=f"msk{b}_{h}_{mp}")
            for j in range(2):
                mt = 2 * mp + j
                d2 = P["psB"].tile([128, H], F32, tag="scr", name="d2")
                nc.tensor.matmul(
                    d2[:], x2s[0:KD + 2, mt * 128:(mt + 1) * 128],
                    x1s[0:KD + 2, hs], start=True, stop=True)
                if dve_mask:
                    nc.vector.tensor_scalar(
                        msk[:, j, :], d2[:], 0.0, 2.0, ALU.is_gt, ALU.mult)
                else:
                    nc.scalar.activation(msk[:, j, :], d2[:], AF.Sign)
            for zq, first in ((zhi, mp == 0), (zlo, False)):
                for ot in range(2):
                    nc.tensor.matmul(
                        py[:, ot, :],
                        zq[:, 2 * mp:2 * mp + 2, ot * 128:(ot + 1) * 128],
                        msk[:], start=first, stop=False,
                        perf_mode=mybir.MatmulPerfMode.DoubleRow)
        for kt in range(C1 // 128):
            for ot in range(2):
                nc.tensor.matmul(
                    py[:, ot, :],
                    self.C["w1bT"][:, kt, ot * 128:(ot + 1) * 128],
                    p1T[:, kt, hs],
                    start=False, stop=(kt == C1 // 128 - 1))
        # psum holds 8*(y1a + y1b): the 1/8 rescale rides the evacuation
        if dve_mask:
            nc.scalar.mul(y1_sb[:, :, hs], py[:], 1.0 / Z8)
        else:
            for ot in range(2):
                nc.scalar.activation(y1_sb[:, ot, hs], py[:, ot, :],
                                     AF.Identity, scale=1.0 / Z8,
                                     bias=cs_sb[:, ot:ot + 1])

    def bn_chunk(self, strip, b, h, y_sb):
        nc = self.nc
        for ot in range(2):
            nc.vector.bn_stats(
                strip[:, ot, (b * NH + h) * 6:(b * NH + h + 1) * 6],
                y_sb[:, ot, h * H:(h + 1) * H])

    def bn_prep(self, strip, name):
        """Aggregate bn_stats into [sum, sumsq] per channel (DVE smalls).

        Emitted immediately after the last stats chunk so the collective's
        inputs never get buried behind the next body's top-8 backlog."""
        nc, P = self.nc, self.P
        NSAMP = float(B_PER_CORE * N)
        arin = P["small"].tile([128, 4], F32, tag=f"{name}_arin")
        for ot in range(2):
            agg = P["small"].tile([128, 2], F32, tag=f"{name}_agg")
            nc.vector.bn_aggr(agg[:], strip[:, ot, :])
            nc.vector.tensor_scalar(arin[:, 2 * ot:2 * ot + 1], agg[:, 0:1],
                                    NSAMP, None, ALU.mult)
            m2 = P["small"].tile([128, 1], F32, tag=f"{name}_m2")
            nc.vector.tensor_tensor(m2[:], agg[:, 0:1], agg[:, 0:1], ALU.mult)
            sq = P["small"].tile([128, 1], F32, tag=f"{name}_sq")
            nc.vector.tensor_tensor(sq[:], agg[:, 1:2], m2[:], ALU.add)
            nc.vector.tensor_scalar(arin[:, 2 * ot + 1:2 * ot + 2], sq[:],
                                    NSAMP, None, ALU.mult)
        return arin

    def bn_fire(self, arin, name):
        """Stage + dispatch the AllReduce (Pool queue only)."""
        nc, P = self.nc, self.P
        din = P["dram"].tile([128, 4], F32, tag=f"{name}_din")
        dout = P["dram"].tile([128, 4], F32, tag=f"{name}_dout")
        nc.gpsimd.dma_start(din[:], arin[:])
        nc.gpsimd.collective_compute(
            "AllReduce", ALU.add, replica_groups=[list(range(N_CORES))],
            ins=[din.opt()], outs=[dout.opt()])
        return dout

    def bn_scale_shift(self, dout, gb_sb, name):
        """s = g*rsqrt(var+eps), t = beta - mean*s; ACT/Pool only (no DVE).

        rsqrt via Exp(-0.5*Ln(x)) + 2 Newton steps (mult-only)."""
        nc, P = self.nc, self.P
        NTOT = float(B_PER_CORE * N) * N_CORES
        ag = P["small"].tile([128, 4], F32, tag=f"{name}_ag")
        nc.gpsimd.dma_start(ag[:], dout[:])
        s_sb = P["small"].tile([128, 2], F32, tag=f"{name}_s")
        t_sb = P["small"].tile([128, 2], F32, tag=f"{name}_t")
        mean = P["small"].tile([128, 2], F32, tag=f"{name}_mean")
        x = P["small"].tile([128, 2], F32, tag=f"{name}_x")
        for ot in range(2):
            nc.vector.tensor_scalar(mean[:, ot:ot + 1],
                                    ag[:, 2 * ot:2 * ot + 1],
                                    1.0 / NTOT, None, ALU.mult)
            ey2 = P["small"].tile([128, 1], F32, tag=f"{name}_ey2")
            nc.vector.tensor_scalar(ey2[:], ag[:, 2 * ot + 1:2 * ot + 2],
                                    1.0 / NTOT, None, ALU.mult)
            m2 = P["small"].tile([128, 1], F32, tag=f"{name}_gm2")
            nc.vector.tensor_tensor(m2[:], mean[:, ot:ot + 1],
                                    mean[:, ot:ot + 1], ALU.mult)
            v = P["small"].tile([128, 1], F32, tag=f"{name}_v")
            nc.vector.tensor_tensor(v[:], ey2[:], m2[:], ALU.subtract)
            nc.vector.tensor_scalar(x[:, ot:ot + 1], v[:], EPS_BN, None,
                                    ALU.add)
        # r0 = exp(-0.5*ln(x)) on ACT (2 tiny ops; ln/exp share the act
        # table with sign/relu/identity so no table reload), refined by
        # Newton on Pool.  Popped only after the AllReduce has landed, so
        # neither queue parks.
        lnx = P["small"].tile([128, 2], F32, tag=f"{name}_lnx")
        nc.scalar.activation(lnx[:], x[:], AF.Ln)
        r = P["small"].tile([128, 2], F32, tag=f"{name}_r0")
        nc.scalar.activation(r[:], lnx[:], AF.Exp, scale=-0.5)
        for it in range(2):
            r2 = P["small"].tile([128, 2], F32, tag=f"{name}_r2_{it}")
            nc.vector.tensor_tensor(r2[:], r[:], r[:], ALU.mult)
            p = P["small"].tile([128, 2], F32, tag=f"{name}_p_{it}")
            nc.vector.tensor_tensor(p[:], x[:], r2[:], ALU.mult)
            q = P["small"].tile([128, 2], F32, tag=f"{name}_q_{it}")
            nc.vector.tensor_scalar(q[:], p[:], -0.5, 1.5, ALU.mult, ALU.add)
            rn = P["small"].tile([128, 2], F32, tag=f"{name}_rn_{it}")
            nc.vector.tensor_tensor(rn[:], r[:], q[:], ALU.mult)
            r = rn
        # s = g * r ; t = beta - mean * s   (gb layout [g0,b0,g1,b1])
        for ot in range(2):
            nc.vector.tensor_tensor(s_sb[:, ot:ot + 1], r[:, ot:ot + 1],
                                    gb_sb[:, 2 * ot:2 * ot + 1], ALU.mult)
            ms = P["small"].tile([128, 1], F32, tag=f"{name}_ms")
            nc.vector.tensor_tensor(ms[:], mean[:, ot:ot + 1],
                                    s_sb[:, ot:ot + 1], ALU.mult)
            nc.vector.tensor_tensor(t_sb[:, ot:ot + 1],
                                    gb_sb[:, 2 * ot + 1:2 * ot + 2], ms[:],
                                    ALU.subtract)
        return s_sb, t_sb

    def make_tail(self, st):
        """Deferred pieces, popped during later iterations' emission.

        main = [s1t1, N/M2/EV2 per (b,h) x16, R2]  (popped next body)
        late = [s2t2, OUT per (b,ot2,oh) x8]       (popped body after next)

        The normalize uses relu(s1*y1+t1) = max(s1*y1, -t1) + t1: the max
        runs as ONE Pool op and the +t1 term collapses into a per-channel
        constant c = W2@t1 added at the y2 PSUM evacuation."""
        nc, P, D, C = self.nc, self.P, self.D, self.C
        B = B_PER_CORE
        pieces = []
        box = {}

        def p_r1():
            box["ar1"] = self.bn_fire(st["ar1in"], "bn1")

        pieces.append(p_r1)

        def p_s1t1():
            s1, t1 = self.bn_scale_shift(box["ar1"], C["gb1"], "bn1")
            # relu(s1*y1+t1) = max(s1*y1, -t1) + t1: the max runs on DVE
            # in 2x mode (fp16 SBUF in/out); +t1 collapses into c = W2@t1
            # applied at the y2 evacuation (fp16 rounding of t1 is a
            # per-channel constant, cancelled exactly by BN2's mean).
            tneg = P["small"].tile([128, 2], F32, tag="bn1_tneg")
            nc.vector.tensor_scalar(tneg[:], t1[:], -1.0, None, ALU.mult)
            t1h = P["small"].tile([128, 2], F16, tag="bn1_t1h")
            nc.vector.tensor_copy(t1h[:], t1[:])
            c_ps = P["psB"].tile([128, 2], F32, tag="scr", name="c_ps")
            for ot2 in range(2):
                for kt in range(2):
                    nc.tensor.matmul(
                        c_ps[:, ot2:ot2 + 1],
                        C["w2T"][:, kt, ot2 * 128:(ot2 + 1) * 128],
                        t1h[:, kt:kt + 1], start=(kt == 0), stop=(kt == 1))
            c_sb = P["small"].tile([128, 2], F32, tag="bn1_c")
            nc.scalar.copy(c_sb[:], c_ps[:])
            box["s1"], box["tneg"], box["c"] = s1, tneg, c_sb

        pieces.append(p_s1t1)

        y2_l = [P["y2big"].tile([128, 2, N], F16, tag="y2", name=f"y2_{i}")
                for i in range(B)]
        bn2_strip = P["stats"].tile([128, 2, B * NH * 6], F32, tag="bn2")

        def mk_nm(b, h):
            def p():
                s1, tneg, c = box["s1"], box["tneg"], box["c"]
                y1_sb = st["y1_l"][b]
                y2_sb = y2_l[b]
                hs = slice(h * H, (h + 1) * H)
                for kt in range(2):
                    nc.vector.tensor_scalar(
                        y1_sb[:, kt, hs], y1_sb[:, kt, hs],
                        s1[:, kt:kt + 1], tneg[:, kt:kt + 1],
                        ALU.mult, ALU.max)
                p2y = P["psY"].tile([128, 2, H], F32, tag="psy")
                for ot2 in range(2):
                    for kt in range(2):
                        nc.tensor.matmul(
                            p2y[:, ot2, :],
                            C["w2T"][:, kt, ot2 * 128:(ot2 + 1) * 128],
                            y1_sb[:, kt, hs],
                            start=(kt == 0), stop=(kt == 1))
                for ot2 in range(2):
                    nc.scalar.activation(
                        y2_sb[:, ot2, hs], p2y[:, ot2, :], AF.Identity,
                        bias=c[:, ot2:ot2 + 1])
            return p

        for b in range(B):
            for h in range(NH):
                pieces.append(mk_nm(b, h))

        # bn2 stats in 4 bundled pieces, popped in the pure phase AFTER
        # the DVE top-8 backlog has drained — keeps the R2 collective's
        # inputs ready the moment it is emitted (no Pool.SEQ parking).
        def mk_st(q):
            def p():
                for i in range(4 * q, 4 * q + 4):
                    b, h = divmod(i, NH)
                    self.bn_chunk(bn2_strip, b, h, y2_l[b])
            return p

        for q in range(4):
            pieces.append(mk_st(q))

        def p_r2():
            box["ar2"] = self.bn_fire(self.bn_prep(bn2_strip, "bn2"),
                                      "bn2")

        pieces.append(p_r2)
        late = []

        def p_s2t2():
            box["s2"], box["t2"] = self.bn_scale_shift(
                box["ar2"], C["gb2"], "bn2")

        late.append(p_s2t2)

        def mk_out(b, ot2, oh):
            def p():
                s2, t2 = box["s2"], box["t2"]
                y2_sb = y2_l[b]
                osl = slice(oh * 1024, (oh + 1) * 1024)
                ot_out = P["ostage"].tile([128, 1024], F32, tag="ost")
                nc.scalar.activation(ot_out[:], y2_sb[:, ot2, osl],
                                     AF.Relu, scale=s2[:, ot2:ot2 + 1],
                                     bias=t2[:, ot2:ot2 + 1])
                nc.sync.dma_start(
                    D["out"].ap()[b][ot2 * 128:(ot2 + 1) * 128, osl],
                    ot_out[:])
            return p

        for b in range(B):
            for ot2 in range(2):
                for oh in range(4):
                    late.append(mk_out(b, ot2, oh))
        return pieces, late


def build_program(dbg=False, repeat=1, timing=False):
    key = ("nc", dbg, repeat, timing)
    if key in _PROGRAM_CACHE:
        return _PROGRAM_CACHE[key]
    nc = bacc.Bacc("TRN2", target_bir_lowering=False, debug=False,
                   num_devices=N_CORES)
    B = B_PER_CORE
    big = "Internal" if timing else "ExternalInput"
    D = {}
    D["x1s"] = nc.dram_tensor("x1s", [B, KROWS, N], F16, kind=big)
    D["x2s"] = nc.dram_tensor("x2s", [B, KROWS, M], F16, kind=big)
    D["p1T"] = nc.dram_tensor("p1T", [B, C1, N], F16, kind=big)
    D["p2T"] = nc.dram_tensor("p2T", [B, C2, M], F16, kind=big)
    zw_d = nc.dram_tensor("zw", [C2, O], F16, kind="ExternalInput")
    w1bT_d = nc.dram_tensor("w1bT", [C1, O], F16, kind="ExternalInput")
    w2T_d = nc.dram_tensor("w2T", [O, O], F16, kind="ExternalInput")
    gb1_d = nc.dram_tensor("gb1", [128, 4], F32, kind="ExternalInput")
    gb2_d = nc.dram_tensor("gb2", [128, 4], F32, kind="ExternalInput")
    ident_d = nc.dram_tensor("ident", [128, 128], F32, kind="ExternalInput")
    D["csb"] = nc.dram_tensor("csb", [B, 128, 2], F32, kind="ExternalInput")
    D["out"] = nc.dram_tensor("out", [B, O, N], F32,
                              kind="Internal" if timing else "ExternalOutput")
    if timing:
        nc.dram_tensor("tout", [128, 2], F32, kind="ExternalOutput")

    with tile.TileContext(nc) as tc:
        with (
            tc.tile_pool(name="consts", bufs=1) as consts,
            tc.tile_pool(name="inp", bufs=2) as inp,
            tc.tile_pool(name="p2pool", bufs=2) as p2pool,
            tc.tile_pool(name="zpool", bufs=2) as zpool,
            tc.tile_pool(name="ybig", bufs=3) as ybig,
            tc.tile_pool(name="y2big", bufs=2) as y2big,
            tc.tile_pool(name="masks", bufs=3) as masks,
            tc.tile_pool(name="small", bufs=2) as small,
            tc.tile_pool(name="stats", bufs=2) as stats,
            tc.tile_pool(name="ostage", bufs=2) as ostage,
            tc.tile_pool(name="dram", bufs=2, space="DRAM") as drampool,
            tc.tile_pool(name="psA", bufs=2, space="PSUM") as psA,
            tc.tile_pool(name="psB", bufs=2, space="PSUM") as psB,
            tc.tile_pool(name="psY", bufs=2, space="PSUM") as psY,
        ):
            P = dict(inp=inp, p2pool=p2pool, zpool=zpool, ybig=ybig,
                     y2big=y2big, masks=masks, small=small, stats=stats,
                     ostage=ostage, dram=drampool, psA=psA, psB=psB,
                     psY=psY)
            # ---- constants ----
            C = {}
            C["zw"] = consts.tile([128, C2 // 128, O], F16, name="zw_sb")
            nc.sync.dma_start(C["zw"][:], zw_d.ap().rearrange(
                "(k p) o -> p k o", p=128))
            C["w1bT"] = consts.tile([128, C1 // 128, O], F16, name="w1bT_sb")
            nc.sync.dma_start(C["w1bT"][:], w1bT_d.ap().rearrange(
                "(k p) o -> p k o", p=128))
            C["w2T"] = consts.tile([128, O // 128, O], F16, name="w2T_sb")
            nc.sync.dma_start(C["w2T"][:], w2T_d.ap().rearrange(
                "(k p) o -> p k o", p=128))
            C["gb1"] = consts.tile([128, 4], F32, name="gb1_sb")
            nc.sync.dma_start(C["gb1"][:], gb1_d.ap())
            C["gb2"] = consts.tile([128, 4], F32, name="gb2_sb")
            nc.sync.dma_start(C["gb2"][:], gb2_d.ap())
            C["ident"] = consts.tile([128, 128], F32, name="ident_sb")
            nc.sync.dma_start(C["ident"][:], ident_d.ap())
            if timing:
                zt = consts.tile([128, 4096], F16)
                nc.gpsimd.memset(zt[:], 0.0)
                for nm in ("x1s", "x2s", "p1T", "p2T"):
                    flat = D[nm].ap().rearrange("a b c -> (a b c)")
                    total = flat.shape[0]
                    csz = 128 * 4096
                    for off in range(0, total, csz):
                        ln = min(csz, total - off)
                        nc.sync.dma_start(
                            flat[off:off + ln].rearrange(
                                "(p f) -> p f", p=128),
                            zt[:, 0:ln // 128])

            pending = {"main": [], "late": []}
            for rep in range(repeat):
                body = _Body(nc, tc, P, D, C, rep)
                pending = body.emit(pending)
            # drain: main (s1t1/NME/R2 of the last body) first — the last
            # late pieces (s2t2/OUT) depend on its R2 collective.
            for p in pending["main"]:
                p()
            for p in pending["late"]:
                p()
    nc.compile()
    _PROGRAM_CACHE[key] = nc
    return nc


def _prep_core(xyz1, xyz2, points1, points2):
    """Host-side prep of one core's 2 batches."""
    B = xyz1.shape[0]
    x1s = np.zeros((B, KROWS, N), np.float16)
    x2s = np.zeros((B, KROWS, M), np.float16)
    for b in range(B):
        s1, s2 = _build_sides(xyz1[b], xyz2[b])
        x1s[b], x2s[b] = s1, s2
    p1T = np.ascontiguousarray(points1.transpose(0, 2, 1)).astype(np.float16)
    p2T = np.ascontiguousarray(points2.transpose(0, 2, 1)).astype(np.float16)
    return x1s, x2s, p1T, p2T


def _csb(p2, zw):
    """colsum of the effective (fp8 hi+lo quantized) Z per batch."""
    import ml_dtypes
    f8 = np.dtype(ml_dtypes.float8_e4m3)
    out = np.zeros((p2.shape[0], 128, 2), np.float32)
    for b in range(p2.shape[0]):
        z = (p2[b].astype(np.float16).astype(np.float32)
             @ zw.astype(np.float32)) * np.float32(Z8)
        hi = z.astype(f8).astype(np.float32)
        lo = (z - hi).astype(f8).astype(np.float32)
        cs = (hi + lo).sum(0) / np.float32(Z8)
        out[b] = cs.reshape(2, 128).T
    return out


def kernel(xyz1, xyz2, points1, points2, W1, b1, g1, beta1, W2, b2, g2,
           beta2):
    xyz1, xyz2 = np.asarray(xyz1), np.asarray(xyz2)
    points1, points2 = np.asarray(points1), np.asarray(points2)
    W1, W2 = np.asarray(W1, np.float32), np.asarray(W2, np.float32)
    g1, beta1 = np.asarray(g1, np.float32), np.asarray(beta1, np.float32)
    g2, beta2 = np.asarray(g2, np.float32), np.asarray(beta2, np.float32)
    # interpolation weight exactly as the reference computes it
    dist = np.float32(1e-10)
    inv = np.float32(1.0) / dist
    ssum = (inv + inv) + inv
    w = inv / ssum  # fp32(1/3)-ish, bit-exact vs reference

    zw = (0.5 * w * W1[:, :C2].astype(np.float32)).T.astype(np.float16)
    w1bT = np.ascontiguousarray(W1[:, C2:].T).astype(np.float32)
    w1bT = (w1bT * np.float32(Z8)).astype(np.float16)
    w2T = np.ascontiguousarray(W2.T).astype(np.float16)
    # conv biases b1/b2 are no-ops through BN (mean subtracts them exactly)
    gb1 = np.stack([g1[0:128], beta1[0:128], g1[128:256], beta1[128:256]],
                   1).astype(np.float32)
    gb2 = np.stack([g2[0:128], beta2[0:128], g2[128:256], beta2[128:256]],
                   1).astype(np.float32)
    ident = np.eye(128, dtype=np.float32)

    nc = build_program()
    in_maps = []
    for c in range(N_CORES):
        bs = slice(c * B_PER_CORE, (c + 1) * B_PER_CORE)
        x1s, x2s, p1T, p2T = _prep_core(
            np.asarray(xyz1[bs]), np.asarray(xyz2[bs]),
            np.asarray(points1[bs]), np.asarray(points2[bs]))
        csb = _csb(np.asarray(points2[bs]), zw)
        in_maps.append(dict(x1s=x1s, x2s=x2s, p1T=p1T, p2T=p2T, zw=zw,
                            w1bT=w1bT, w2T=w2T, gb1=gb1, gb2=gb2,
                            ident=ident, csb=csb))
    res = bass_utils.run_bass_kernel_spmd(
        nc, in_maps, core_ids=list(range(N_CORES)), trace=False)
    out = np.concatenate([res.results[c]["out"] for c in range(N_CORES)],
                         axis=0)
    return out.astype(np.float32)
